# revision 2
# baseline (speedup 1.0000x reference)
"""Trainium2 Bass kernel for CustomMultiheadAttention (linear attention with
low-rank QKV projections) — token-sharded version.

Math (fp32 reference):
    q = elu(query @ Wq.T + q_up_b) + 1      with Wq = q_up_w @ q_down_w
    k = elu(key   @ Wk.T + k_up_b) + 1
    v =      value @ Wv.T + v_up_b
    per head h (16 heads, head_dim 64):
        kv_h    = k_h^T v_h                  # [64, 64]
        ksum_h  = sum_t k_h[t]               # [64]
        num     = q_h kv_h                   # [S, 64]
        denom   = q_h . ksum_h               # [S]
        attn_h  = num / (denom + 1e-6)
    out = concat_h(attn_h) @ out_w.T + out_b

Sharding: 8 cores = 4 batches x 2 token-halves. Each core projects q/k/v for
its 2048 tokens across ALL 16 heads (RANK==E/2, so folding down+up into one
[E,E] weight is FLOP-neutral and kills the dT intermediate). kv/ksum partial
sums (one [128, 1040] f32 buffer) are AllReduced between the two cores of a
batch; everything after (num, denom, out proj) is token-local. Host
concatenates the 8 [2048, 1024] output tiles and adds out_b.

Overlap plan: the q projections for all 4 chunks are emitted before anything
that depends on the AllReduce, and the whole AR-dependent chain (psum
staging -> bounce DMA -> AR -> readback -> kv2/kblk builds) runs on the
otherwise-idle scalar+gpsimd engines, so the PE streams q projections while
the collective completes. attn reuses the qT tiles in place (num overwrites
qT after its last matmul read).

Device layouts (per core; matmul operands float32r, full-rate PE):
    xq/xk/xv  [E=1024, T=2048]  inputs pre-transposed on host (token-minor)
    k/v proj: x-tiles stationary, W columns moving  -> feat[t, j] (token-major)
    q proj:   W-tiles stationary, x moving          -> qT[j, t] (dim-major)
    kv[128, 128] blocks per 2-head pair, psum-resident over all 4 chunks
    (banks DVE-memset to open the episode; all block matmuls accumulate with
    start=False — start=True would clear has_written for the WHOLE bank and
    wipe sibling regions).
    k bias folded in via a rank-1 (ones x bias-row) matmul; q bias via the
    ACT bias port; elu+1 computed as max(u,0) + min(exp(u),1).
"""

import numpy as np

import concourse.bass as bass  # noqa: F401
import concourse.mybir as mybir
import concourse.tile as tile
from concourse import bacc
from concourse.bass_utils import run_bass_kernel_spmd

F32 = mybir.dt.float32
F32R = mybir.dt.float32r
BF16 = mybir.dt.bfloat16
AF = mybir.ActivationFunctionType
OP = mybir.AluOpType

P = 128          # partitions
E = 1024         # embed dim
S = 4096         # sequence length
B = 4            # batch
T = S // 2       # tokens per core
TC = 512         # token chunk
NCHUNK = T // TC  # 4
NE = E // P      # 8 e-tiles (contraction)
NJ = E // P      # 8 j-tiles (all 16 heads)
NTS = TC // P    # 4 token subtiles per chunk
NH = 16          # heads
DH = 64          # head dim

_CACHE = {}


def _build():
    nc = bacc.Bacc(None, target_bir_lowering=False, num_devices=8)

    dp = nc.declare_dram_parameter
    xq = dp("xq", [E, T], BF16, isOutput=False)
    xk = dp("xk", [E, T], BF16, isOutput=False)
    xv = dp("xv", [E, T], BF16, isOutput=False)
    wq = dp("wq", [E, E], BF16, isOutput=False)
    wk = dp("wk", [E, E], BF16, isOutput=False)
    wv = dp("wv", [E, E], BF16, isOutput=False)
    wo = dp("wo", [E, E], BF16, isOutput=False)
    bqt = dp("bqt", [P, NJ], F32, isOutput=False)       # q bias, [128,8] tiles
    bkb = dp("bkb", [P, E], BF16, isOutput=False)       # k bias bcast
    bvb = dp("bvb", [P, E], BF16, isOutput=False)       # v bias bcast
    rtm = dp("rtm", [P, NJ * NH], F32, isOutput=False)  # R^T head mask tiles
    r16 = dp("r16", [NH, E], BF16, isOutput=False)      # head-replication mask
    bdm = dp("bdm", [P, NJ * P], BF16, isOutput=False)  # block-diag 0/1 mask
    out_t = dp("out", [T, E], F32, isOutput=True)

    NRED = NJ * (P + 2)  # 1040: 8 x [128 kv cols + ksum col + pad]

    with tile.TileContext(nc) as tcx:
        from contextlib import ExitStack

        with ExitStack() as root:
            cpool = root.enter_context(tcx.tile_pool(name="consts", bufs=1))
            dram = root.enter_context(
                tcx.tile_pool(name="dram", bufs=1, space="DRAM"))
            rtm_sb = cpool.tile([P, NJ * NH], F32)
            r16_sb = cpool.tile([NH, E], BF16)
            bqt_sb = cpool.tile([P, NJ], F32)
            bkb_sb = cpool.tile([P, E], BF16)
            bvb_sb = cpool.tile([P, E], BF16)
            kv2 = cpool.tile([P, NJ, P], BF16)   # block-diag kv pairs
            kblk = cpool.tile([P, NJ * NH], BF16)
            rb = cpool.tile([P, NRED], F32)      # reduce stage / readback
            bdm_sb = cpool.tile([P, NJ * P], BF16)
            wq_sb = cpool.tile([P, NE, E], BF16)
            ib = dram.tile([P, NRED], F32)
            ob = dram.tile([P, NRED], F32)

            nc.sync.dma_start(out=bkb_sb[:], in_=bkb[:])
            nc.sync.dma_start(out=rtm_sb[:], in_=rtm[:])
            nc.sync.dma_start(out=r16_sb[:], in_=r16[:])
            nc.sync.dma_start(out=bqt_sb[:], in_=bqt[:])
            nc.sync.dma_start(out=bvb_sb[:], in_=bvb[:])
            nc.sync.dma_start(out=bdm_sb[:], in_=bdm[:])

            # ---------------- Phase KV ----------------
            with ExitStack() as ph:
                wpool = ph.enter_context(tcx.tile_pool(name="wkv", bufs=1))
                wk_sb = wpool.tile([P, NE, E], BF16)
                wv_sb = wpool.tile([P, NE, E], BF16)
                # DMA service follows emission order, and the first psum
                # group only reads one e-tile at a time: per-et slices let
                # the first matmuls start after ~1/8 of the weight load.
                xpool = ph.enter_context(tcx.tile_pool(name="xkv", bufs=2))
                wk_r = wk.rearrange("(a p) j -> p a j", p=P)
                xk0_r = xk[:, 0:TC].rearrange("(a p) t -> p a t", p=P)
                xtk0 = xpool.tile([P, NE, TC], BF16, tag="x", name="xk")
                for et in range(NE):
                    nc.sync.dma_start(out=wk_sb[:, et, :], in_=wk_r[:, et, :])
                    nc.sync.dma_start(out=xtk0[:, et, :], in_=xk0_r[:, et, :])
                kfpool = ph.enter_context(tcx.tile_pool(name="kf", bufs=2))
                vcpool = ph.enter_context(tcx.tile_pool(name="vc", bufs=1))
                tpool = ph.enter_context(tcx.tile_pool(name="tkv", bufs=2))
                psp = ph.enter_context(
                    tcx.tile_pool(name="psp", bufs=4, space="PSUM"))
                psk = ph.enter_context(
                    tcx.tile_pool(name="psk", bufs=1, space="PSUM"))

                # kv blocks are [128, 130]: 128 cols of k^T v plus a ones
                # column producing ksum for free. 3 regions per psum bank.
                kvps = [psk.tile([P, 3 if i < 2 else 2, P + 2], F32,
                                 name=f"kvps{i}") for i in range(3)]
                # DVE memset opens each bank's accumulation episode: values 0,
                # and whatever the has_written bits are, the first start=False
                # matmul per element does the right thing (accumulate onto 0
                # or overwrite-with-set). Also gives every block matmul a WAW
                # dep on the memset, forcing episode ordering.
                for kb in kvps:
                    nc.vector.memset(kb[:], 0.0)

                for ci in range(NCHUNK):
                    # ---- k projection + feature map ----
                    if ci == 0:
                        xtk = xtk0
                        nc.sync.dma_start(
                            out=wv_sb[:],
                            in_=wv.rearrange("(a p) j -> p a j", p=P))
                    else:
                        xtk = xpool.tile([P, NE, TC], BF16, tag="x",
                                         name="xk")
                        nc.sync.dma_start(
                            out=xtk[:],
                            in_=xk[:, ci * TC:(ci + 1) * TC].rearrange(
                                "(a p) t -> p a t", p=P))
                    kfeat = kfpool.tile([P, NTS, E], BF16, tag="kf",
                                        name="kfeat")
                    for ts in range(NTS):
                        for half in range(2):
                            js = slice(half * TC, (half + 1) * TC)
                            pu = psp.tile([P, TC], F32, tag="pp", name="puk")
                            for et in range(NE):
                                nc.tensor.matmul(
                                    pu[:], xtk[:, et, ts * P:(ts + 1) * P],
                                    wk_sb[:, et, js],
                                    start=(et == 0), stop=(et == NE - 1),
                                )
                            # feat = max(u,0) + min(exp(u),1), u = pu + bk
                            uu = tpool.tile([P, TC], F32, tag="uu", name="uu")
                            u0 = tpool.tile([P, TC], F32, tag="u0", name="u0")
                            ex = tpool.tile([P, TC], F32, tag="ex", name="ex")
                            nc.vector.tensor_tensor(
                                uu[:], pu[:], bkb_sb[:, js], op=OP.add)
                            nc.vector.tensor_scalar_max(u0[:], uu[:], 0.0)
                            nc.scalar.activation(ex[:], uu[:], AF.Exp)
                            nc.vector.scalar_tensor_tensor(
                                kfeat[:, ts, js], ex[:], 1.0, u0[:],
                                op0=OP.min, op1=OP.add,
                            )

                    # ---- v projection ----
                    xtv = xpool.tile([P, NE, TC], BF16, tag="x", name="xv")
                    nc.sync.dma_start(
                        out=xtv[:],
                        in_=xv[:, ci * TC:(ci + 1) * TC].rearrange(
                            "(a p) t -> p a t", p=P))
                    if ci == 1:
                        nc.sync.dma_start(
                            out=wq_sb[:],
                            in_=wq.rearrange("(a p) j -> p a j", p=P))
                    if ci == 0:
                        vch = vcpool.tile([P, NTS, NJ, P + 2], BF16,
                                          tag="vc", name="vch")
                        # ones columns [.., 128:130] persist across chunks
                        nc.vector.memset(vch[:], 1.0)
                    for ts in range(NTS):
                        for half in range(2):
                            js = slice(half * TC, (half + 1) * TC)
                            g = slice(half * 4, (half + 1) * 4)
                            pv = psp.tile([P, TC], F32, tag="pp", name="puv")
                            for et in range(NE):
                                nc.tensor.matmul(
                                    pv[:], xtv[:, et, ts * P:(ts + 1) * P],
                                    wv_sb[:, et, js],
                                    start=(et == 0), stop=(et == NE - 1),
                                )
                            nc.vector.tensor_tensor(
                                vch[:, ts, g, 0:P],
                                pv[:].rearrange("p (a b) -> p a b", a=4),
                                bvb_sb[:, js].rearrange(
                                    "p (a b) -> p a b", a=4),
                                op=OP.add)

                    # ---- kv blocks + ksum (psum-resident accumulation) ----
                    for ts in range(NTS):
                        last = ci == NCHUNK - 1 and ts == NTS - 1
                        for j1 in range(NJ):
                            nc.tensor.matmul(
                                kvps[j1 // 3][:, j1 % 3, :],
                                kfeat[:, ts, j1 * P:(j1 + 1) * P],
                                vch[:, ts, j1, :],
                                start=False, stop=last,
                                skip_group_check=True,
                            )

                # stage partials for the pairwise AllReduce (scalar engine:
                # the DVE/sync queues stay free for the q projections)
                for i in range(3):
                    n = 3 if i < 2 else 2
                    nc.scalar.copy(
                        rb[:, i * 3 * (P + 2):(i * 3 + n) * (P + 2)]
                        .rearrange("p (a b) -> p a b", a=n),
                        kvps[i][:])

            # whole AR chain lives on the gpsimd queue
            nc.gpsimd.dma_start(out=ib[:], in_=rb[:])
            nc.gpsimd.collective_compute(
                "AllReduce", mybir.AluOpType.add,
                replica_groups=[[0, 1], [2, 3], [4, 5], [6, 7]],
                ins=[ib.opt()], outs=[ob.opt()],
            )
            nc.gpsimd.dma_start(out=rb[:], in_=ob[:])

            # kv2 block-diag + kblk denominator masks (gpsimd: keeps DVE free)
            rb_blocks = rb[:].rearrange("p (a b) -> p a b", a=NJ)
            nc.gpsimd.tensor_tensor(
                kv2[:], rb_blocks[:, :, 0:P],
                bdm_sb[:].rearrange("p (a b) -> p a b", a=NJ), op=OP.mult)
            for j1 in range(NJ):
                nc.gpsimd.tensor_scalar(
                    kblk[:, j1 * NH:(j1 + 1) * NH],
                    rtm_sb[:, j1 * NH:(j1 + 1) * NH],
                    rb[:, j1 * (P + 2) + P:j1 * (P + 2) + P + 1],
                    None, op0=OP.mult,
                )

            # ---------------- Phase Q + output ----------------
            with ExitStack() as ph:
                wpool = ph.enter_context(tcx.tile_pool(name="wq2", bufs=1))
                wo_sb = wpool.tile([P, NJ, E], BF16)
                dpl = wpool.tile([NH, TC], F32)
                rcp = wpool.tile([NH, TC], BF16)

                xpool = ph.enter_context(tcx.tile_pool(name="xqp", bufs=2))
                qpool = ph.enter_context(tcx.tile_pool(name="qf", bufs=4))
                tpool = ph.enter_context(tcx.tile_pool(name="tq", bufs=2))
                opool = ph.enter_context(tcx.tile_pool(name="osb", bufs=2))
                psq = ph.enter_context(
                    tcx.tile_pool(name="psq", bufs=2, space="PSUM"))
                psd = ph.enter_context(
                    tcx.tile_pool(name="psd", bufs=2, space="PSUM"))
                psn = ph.enter_context(
                    tcx.tile_pool(name="psn", bufs=2, space="PSUM"))
                pso = ph.enter_context(
                    tcx.tile_pool(name="pso", bufs=2, space="PSUM"))

                def emit_qproj(ci):
                    xtq = xpool.tile([P, NE, TC], BF16, tag="x", name="xq")
                    nc.sync.dma_start(
                        out=xtq[:],
                        in_=xq[:, ci * TC:(ci + 1) * TC].rearrange(
                            "(a p) t -> p a t", p=P))
                    qT = qpool.tile([P, NJ, TC], BF16, tag="qT", name="qT")
                    for jt in range(NJ):
                        pq = psq.tile([P, TC], F32, tag="pq", name="pq")
                        for et in range(NE):
                            nc.tensor.matmul(
                                pq[:], wq_sb[:, et, jt * P:(jt + 1) * P],
                                xtq[:, et, :],
                                start=(et == 0), stop=(et == NE - 1),
                            )
                        bq_ap = bqt_sb[:, jt:jt + 1]
                        u0 = tpool.tile([P, TC], F32, tag="qu", name="qu")
                        ex = tpool.tile([P, TC], F32, tag="qe", name="qe")
                        nc.vector.tensor_scalar(
                            u0[:], pq[:], bq_ap, 0.0, op0=OP.add, op1=OP.max)
                        nc.scalar.activation(ex[:], pq[:], AF.Exp, bias=bq_ap)
                        nc.vector.scalar_tensor_tensor(
                            qT[:, jt, :], ex[:], 1.0, u0[:],
                            op0=OP.min, op1=OP.add,
                        )
                    return qT

                # all q projections run while the AllReduce completes
                qTs = {ci: emit_qproj(ci) for ci in range(NCHUNK)}

                # wo after the xq chunks: needed only at out-proj time
                nc.sync.dma_start(
                    out=wo_sb[:], in_=wo.rearrange("(a p) o -> p a o", p=P))

                def emit_denom_num(ci, qT):
                    # denom^T[h, t] then reciprocal on DVE while num runs;
                    # num overwrites qT in place (last matmul read was here)
                    pdn = psd.tile([NH, TC], F32, tag="pdn", name="pdn")
                    for jt in range(NJ):
                        nc.tensor.matmul(
                            pdn[:], kblk[:, jt * NH:(jt + 1) * NH],
                            qT[:, jt, :],
                            start=(jt == 0), stop=(jt == NJ - 1),
                        )
                    nc.vector.tensor_scalar_add(dpl[:], pdn[:], 1e-6)
                    with nc.allow_low_precision(
                            reason="f32r is f32-width; rep matmul needs f32r"):
                        nc.vector.reciprocal(rcp[:], dpl[:])
                    for jt in range(NJ):
                        pnm = psn.tile([P, TC], F32, tag="pnm", name="pnm")
                        nc.tensor.matmul(
                            pnm[:], kv2[:, jt, :], qT[:, jt, :],
                            start=True, stop=True,
                        )
                        nc.scalar.copy(qT[:, jt, :], pnm[:])
                    return qT

                def emit_attn_out(ci, attn):
                    for jt in range(NJ):
                        prp = pso.tile([P, TC], F32, tag="pso", name="prp")
                        nc.tensor.matmul(
                            prp[:], r16_sb[:, jt * P:(jt + 1) * P], rcp[:],
                            start=True, stop=True,
                        )
                        nc.vector.tensor_tensor(
                            attn[:, jt, :], attn[:, jt, :], prp[:],
                            op=OP.mult)
                    for ts in range(NTS):
                        ob_t = opool.tile([P, 2, TC], F32, tag="ob", name="ob")
                        for oc in range(2):
                            po = pso.tile([P, TC], F32, tag="pso", name="po")
                            for jt in range(NJ):
                                nc.tensor.matmul(
                                    po[:], attn[:, jt, ts * P:(ts + 1) * P],
                                    wo_sb[:, jt, TC * oc:TC * (oc + 1)],
                                    start=(jt == 0), stop=(jt == NJ - 1),
                                )
                            nc.scalar.copy(ob_t[:, oc, :], po[:])
                        row0 = ci * TC + ts * P
                        nc.sync.dma_start(
                            out=out_t[row0:row0 + P, :].rearrange(
                                "p (a b) -> p a b", a=2),
                            in_=ob_t[:],
                        )

                for ci in range(NCHUNK):
                    attn = emit_denom_num(ci, qTs.pop(ci))
                    emit_attn_out(ci, attn)

    nc.compile()
    return nc


def _get_nc():
    if "nc" not in _CACHE:
        _CACHE["nc"] = _build()
    return _CACHE["nc"]


def kernel(**inputs):
    query = np.asarray(inputs["query"], dtype=np.float32)
    key = np.asarray(inputs["key"], dtype=np.float32)
    value = np.asarray(inputs["value"], dtype=np.float32)

    f32 = np.float32

    import ml_dtypes
    bf16 = ml_dtypes.bfloat16

    def comb(up, down):
        # y = x @ down.T @ up.T = x @ (up @ down).T; device wants [e_in, e_out]
        return np.ascontiguousarray(
            (np.asarray(up, f32) @ np.asarray(down, f32)).T.astype(bf16))

    shared = {
        "wq": comb(inputs["q_up_w"], inputs["q_down_w"]),
        "wk": comb(inputs["k_up_w"], inputs["k_down_w"]),
        "wv": comb(inputs["v_up_w"], inputs["v_down_w"]),
        "wo": np.ascontiguousarray(
            np.asarray(inputs["out_w"], f32).T.astype(bf16)),
        "bqt": np.ascontiguousarray(
            np.asarray(inputs["q_up_b"], f32).reshape(NJ, P).T),
    }
    shared["bkb"] = np.ascontiguousarray(np.broadcast_to(
        np.asarray(inputs["k_up_b"], bf16), (P, E)))
    shared["bvb"] = np.ascontiguousarray(np.broadcast_to(
        np.asarray(inputs["v_up_b"], bf16), (P, E)))
    heads = np.arange(E) // DH
    rt_full = (heads[:, None] == np.arange(NH)[None, :]).astype(f32)  # [E,16]
    shared["rtm"] = np.ascontiguousarray(
        rt_full.reshape(NJ, P, NH).transpose(1, 0, 2).reshape(P, NJ * NH))
    shared["r16"] = np.ascontiguousarray(rt_full.T.astype(bf16))      # [16,E]
    # bdm[p, j1*128+c] = 1 where row-dim p and col-dim c are the same head
    half_ = (np.arange(P) // DH)[:, None] == (np.arange(P) // DH)[None, :]
    shared["bdm"] = np.ascontiguousarray(
        np.tile(half_, (1, NJ)).astype(bf16))

    in_maps = []
    for c in range(8):
        b, half = divmod(c, 2)
        tsl = slice(half * T, (half + 1) * T)
        im = dict(shared)
        im["xq"] = np.ascontiguousarray(query[b].T[:, tsl].astype(bf16))
        im["xk"] = np.ascontiguousarray(key[b].T[:, tsl].astype(bf16))
        im["xv"] = np.ascontiguousarray(value[b].T[:, tsl].astype(bf16))
        in_maps.append(im)

    nc = _get_nc()
    # the first execution after a device wedge occasionally dies with
    # NRT_EXEC_UNIT_UNRECOVERABLE; a retry on a clean session recovers
    last_err = None
    for _attempt in range(3):
        try:
            res = run_bass_kernel_spmd(nc, in_maps, core_ids=list(range(8)),
                                       **_CACHE.get("run_kwargs", {}))
            last_err = None
            break
        except Exception as e:  # noqa: BLE001
            last_err = e
            import time
            time.sleep(10)
    if last_err is not None:
        raise last_err
    _CACHE["last_result"] = res

    out_b = np.asarray(inputs["out_b"], dtype=np.float32)
    out = np.empty((B, S, E), np.float32)
    for c in range(8):
        b, half = divmod(c, 2)
        out[b, half * T:(half + 1) * T] = res.results[c]["out"] + out_b
    return out
